# revision 1
# baseline (speedup 1.0000x reference)
"""ChebConv (K=3) forward as a distributed Bass/Tile kernel on 8 trn2 NeuronCores.

Sharding (per spec hint): vertices V are sharded across the 8 cores.
  x0 = [x[0] | x[1]]            # [V, 128], feature col = b*64 + fin
  x1 = L @ x0                   # SpMM (COO, edge-parallel)
  x2' = L @ x1 - 0.5 x0         # = x2/2; the 2x is folded into W_k2
  out[b,v,:] = bias + sum_k xk[v, b*64:(b+1)*64] @ Wk'

Each core owns a row shard (V/8 rows padded to a multiple of 128 = "blocks").
SpMM per core and per 128-edge tile (all data bf16, PSUM accumulate f32):
  - gpsimd.dma_gather fetches the 256B source feature rows from per-piece
    tables (int16 indices), spread round-robin over 4 SWDGE queues.
  - The selector M[e,j] = val[e] * (lrow[e]==j) is built ON-CHIP per tile
    with one fused tensor_scalar (iota==lrow)*val from a 4B/edge
    (lrow,val) stream, alternating DVE / GpSimd engines.
  - PE matmul M^T @ G (spmm1, row-major out) or G^T @ M (spmm2, transposed
    out) performs the scaled segmented sum into a per-block PSUM accumulator.

The column space is split into npc "pieces" = block-ranges sliced ACROSS all
cores (piece j = blocks [B_j, B_j+1) of every core's shard, core-major).
Phase 1 computes x1 blocks in increasing order, and the AllGather of piece j
is issued as soon as its last block is stored, so the collective overlaps
phase 1 and phase 2's gathers (which consume pieces in arrival order) start
immediately. The final channel mix is fused into the SpMM2 block loop using
block-diagonal weights plus a rank-1 bias matmul; -0.5 x0 enters SpMM2's
accumulation as a (-0.5 I) matmul.

The tile structure is computed from the actual edge data at call time (max
over cores per (block, piece) slot) so one SPMD program fits all 8 cores.
"""

import sys

sys.path.insert(0, "/opt/trn_rl_repo")

import numpy as np
import ml_dtypes

import concourse.bass as bass
import concourse.bacc as bacc
import concourse.mybir as mybir
import concourse.tile as tile
from concourse import bass_utils
from concourse.alu_op_type import AluOpType

P = 128
F32 = mybir.dt.float32
BF16 = mybir.dt.bfloat16
I16 = mybir.dt.int16
NPBF16 = ml_dtypes.bfloat16
NQ = 4  # SWDGE queues (parallel descriptor feed; ucode max)


def _cdiv(a, b):
    return -(-a // b)


# ---------------------------------------------------------------------------
# Host-side: uniform (cross-core) edge structure + per-core content arrays
# ---------------------------------------------------------------------------


class EdgeStructure:
    def __init__(self, V, ncores, sb_blocks, npc, rows, cols):
        assert V % ncores == 0
        self.V, self.ncores = V, ncores
        self.vsh = V // ncores
        self.nblk = _cdiv(self.vsh, P)
        self.vpad = self.nblk * P
        self.vtot = self.vpad * ncores

        # pieces: block ranges of every core's shard, sliced across cores.
        # piece j table = [ncores * Rj * P rows]; must fit int16 indices.
        # Sizes tuned so per-(block,piece) edge counts land just UNDER a
        # multiple of 128 (less ceil-quantization padding).
        if self.nblk == 98 and npc == 4:
            Rj = [28, 28, 21, 21]
        else:
            base = self.nblk // npc
            rem = self.nblk - base * npc
            Rj = [base + (1 if j < rem else 0) for j in range(npc)]
        assert all(r * ncores * P <= 32768 for r in Rj)
        self.npc = npc
        self.piece_blocks = Rj
        self.piece_b0 = np.concatenate(([0], np.cumsum(Rj)))  # block bounds
        self.piece_rows = [r * P for r in Rj]

        rows = np.asarray(rows, np.int64)
        cols = np.asarray(cols, np.int64)
        c_of = cols // self.vsh
        l_of = cols - c_of * self.vsh
        cblk = l_of // P
        piece = np.searchsorted(self.piece_b0, cblk, side="right") - 1
        # index within piece table: c * Rj*P + (l - B_j*P)
        pr = np.array(self.piece_rows)[piece]
        self.idx_in_piece = c_of * pr + (l_of - self.piece_b0[piece] * P)
        assert self.idx_in_piece.max() < 32768

        r_core = rows // self.vsh
        r_loc = rows - r_core * self.vsh
        blk = r_loc // P
        nchunks = npc

        # slot order: for sb: for chunk(piece): for block in sb
        sb_arr = blk // sb_blocks
        bi_arr = blk % sb_blocks
        bh_arr = np.minimum(sb_blocks, self.nblk - sb_arr * sb_blocks)
        sid = sb_arr * sb_blocks * nchunks + piece * bh_arr + bi_arr

        self.sb_blocks = sb_blocks
        self.nsb = _cdiv(self.nblk, sb_blocks)
        order = []
        for sb in range(self.nsb):
            b0 = sb * sb_blocks
            bh = min(sb_blocks, self.nblk - b0)
            for ch in range(nchunks):
                for bi in range(bh):
                    order.append((b0 + bi, ch))
        self.nslots = len(order)
        self.slot_block = np.array([b for b, _ in order], np.int64)
        self.slot_chunk = np.array([c for _, c in order], np.int64)

        counts = np.zeros((ncores, self.nslots), np.int64)
        np.add.at(counts, (r_core, sid), 1)
        T = _cdiv(np.max(counts, axis=0), P)

        # every block needs >=1 tile so its PSUM accumulator gets written
        blk_tiles = np.zeros(self.nblk, np.int64)
        np.add.at(blk_tiles, self.slot_block, T)
        for b in np.nonzero(blk_tiles == 0)[0]:
            sb, bi = b // sb_blocks, b % sb_blocks
            bh = min(sb_blocks, self.nblk - sb * sb_blocks)
            T[sb * sb_blocks * nchunks + 0 * bh + bi] = 1

        self.T = T
        self.slot_tile_base = np.concatenate(([0], np.cumsum(T)))[:-1]
        self.ntiles = int(np.sum(T))
        self.sid_of_edge = sid
        self.r_core_of_edge = r_core
        self.lrow_of_edge = (r_loc % P).astype(np.int64)

        # (sb, chunk) -> contiguous tile run
        self.runs = []  # per sb: list of (tile_start, ntiles, chunk)
        s = 0
        for sb in range(self.nsb):
            b0 = sb * sb_blocks
            bh = min(sb_blocks, self.nblk - b0)
            sb_runs = []
            for ch in range(nchunks):
                t0 = int(self.slot_tile_base[s])
                ntr = int(np.sum(T[s : s + bh]))
                if ntr > 0:
                    sb_runs.append((t0, ntr, ch))
                s += bh
            self.runs.append(sb_runs)
        self.max_run_tiles = max(
            nt for sb_runs in self.runs for _, nt, _ in sb_runs
        )

        tile_block = np.empty(self.ntiles, np.int64)
        for s in range(self.nslots):
            t0, ntr = self.slot_tile_base[s], T[s]
            tile_block[t0 : t0 + ntr] = self.slot_block[s]
        self.tile_block = tile_block
        self.tile_start = np.zeros(self.ntiles, bool)
        self.tile_stop = np.zeros(self.ntiles, bool)
        first, last = {}, {}
        for t in range(self.ntiles):
            b = int(tile_block[t])
            if b not in first:
                first[b] = t
            last[b] = t
        for t in first.values():
            self.tile_start[t] = True
        for t in last.values():
            self.tile_stop[t] = True

        # last superblock index per piece (for AllGather issue points)
        self.piece_last_sb = [
            (int(self.piece_b0[j + 1]) - 1) // sb_blocks for j in range(npc)
        ]

    def run_list(self):
        out = []
        for sb_runs in self.runs:
            out.extend(sb_runs)
        return out

    def per_core_arrays(self, core, vals):
        """idx (int16 wrapped+replicated) and bf16 M tiles for one core."""
        sel = np.nonzero(self.r_core_of_edge == core)[0]
        sid = self.sid_of_edge[sel]
        o = np.argsort(sid, kind="stable")
        sel, sid = sel[o], sid[o]
        start = np.searchsorted(sid, np.arange(self.nslots))
        rank = np.arange(len(sid)) - start[sid]
        pos = self.slot_tile_base[sid] * P + rank
        n = self.ntiles * P
        idx = np.zeros(n, np.int16)
        idx[pos] = self.idx_in_piece[sel].astype(np.int16)
        idx_w = np.tile(np.ascontiguousarray(idx.reshape(-1, 16).T), (8, 1))
        m = np.zeros((self.ntiles, P, P), np.float32)
        m[pos // P, pos % P, self.lrow_of_edge[sel]] = vals[sel]
        mfull = np.ascontiguousarray(
            m.astype(NPBF16).transpose(1, 0, 2).reshape(P, self.ntiles * P)
        )
        return idx_w, mfull


# ---------------------------------------------------------------------------
# Bass program (SPMD: one program, per-core data via in_maps)
# ---------------------------------------------------------------------------


def build_program(es: EdgeStructure):
    nblk, vpad, ncores = es.nblk, es.vpad, es.ncores
    nt, GW, SB, npc = es.ntiles, es.max_run_tiles, es.sb_blocks, es.npc

    nc = bacc.Bacc(
        "TRN2",
        target_bir_lowering=False,
        debug=False,
        num_devices=ncores,
        num_swdge_queues=NQ,
    )

    x0p = [
        nc.dram_tensor(f"x0p{j}", [ncores * es.piece_rows[j], P], BF16,
                       kind="ExternalInput")
        for j in range(npc)
    ]
    x0t = nc.dram_tensor("x0t", [nblk, P, P], BF16, kind="ExternalInput")
    wbd = nc.dram_tensor("wbd", [3, P, P], BF16, kind="ExternalInput")
    ident_d = nc.dram_tensor("ident", [P, P], BF16, kind="ExternalInput")
    eidx = nc.dram_tensor("eidx", [P, nt * 8], I16, kind="ExternalInput")
    emt = nc.dram_tensor("emt", [P, nt * P], BF16, kind="ExternalInput")
    outp = nc.dram_tensor("outp", [vpad, P], BF16, kind="ExternalOutput")

    x1my = [
        nc.dram_tensor(f"x1my{j}", [es.piece_rows[j], P], BF16)
        for j in range(npc)
    ]
    x1p = [
        nc.dram_tensor(f"x1p{j}", [ncores * es.piece_rows[j], P], BF16)
        for j in range(npc)
    ]

    with tile.TileContext(nc) as tc:
        with (
            tc.tile_pool(name="const", bufs=1) as cpool,
            tc.tile_pool(name="gslab", bufs=16) as gpool,
            tc.tile_pool(name="mslab", bufs=12) as mpool,
            tc.tile_pool(name="xio", bufs=4) as xpool,
            tc.tile_pool(name="ostage", bufs=6) as opool,
            tc.tile_pool(name="acc", bufs=2 * SB, space="PSUM") as apool,
            tc.tile_pool(name="ptr", bufs=1, space="PSUM") as ptpool,
            tc.tile_pool(name="pmix", bufs=1, space="PSUM") as pmpool,
        ):
            ident_s = cpool.tile([P, P], BF16, tag="ident")
            nc.sync.dma_start(out=ident_s[:], in_=ident_d[:, :])
            wbd_s = cpool.tile([P, 3 * P], BF16, tag="wbd")
            for k in range(3):
                nc.sync.dma_start(
                    out=wbd_s[:, k * P : (k + 1) * P], in_=wbd[k, :, :]
                )
            x1t_s = cpool.tile([P, nblk * P], BF16, tag="x1t")
            eidx_s = cpool.tile([P, nt * 8], I16, tag="eidx")
            nc.sync.dma_start(out=eidx_s[:], in_=eidx[:, :])

            qn = [0]

            def spmm(src_tabs, layout_b, out_cb, after_run=None,
                     pre_sb=None):
                for sb in range(es.nsb):
                    b0 = sb * SB
                    bh = min(SB, nblk - b0)
                    pre = pre_sb(sb, b0, bh) if pre_sb is not None else None
                    psums = {
                        b0 + bi: apool.tile(
                            [P, P], F32, tag="acc", name=f"acc{b0 + bi}"
                        )
                        for bi in range(bh)
                    }
                    for ri, (t0, ntr, ch) in enumerate(es.runs[sb]):
                        mt = mpool.tile([P, GW * P], BF16, tag="m")
                        nc.sync.dma_start(
                            out=mt[:, : ntr * P],
                            in_=emt[:, t0 * P : (t0 + ntr) * P],
                        )
                        g = gpool.tile([P, GW * P], BF16, tag="g")
                        nidx = ntr * P
                        nc.gpsimd.dma_gather(
                            out_ap=g[:, :nidx].rearrange(
                                "p (t e) -> p t e", e=P
                            ),
                            in_ap=src_tabs[ch][:, :],
                            idxs_ap=eidx_s[:, 8 * t0 : 8 * (t0 + ntr)],
                            num_idxs=nidx,
                            num_idxs_reg=nidx,
                            elem_size=P,
                            single_packet=False,
                            queue_num=qn[0] % NQ,
                        )
                        qn[0] += 1
                        for tt in range(ntr):
                            t = t0 + tt
                            b = int(es.tile_block[t])
                            gt = g[:, tt * P : (tt + 1) * P]
                            mm = mt[:, tt * P : (tt + 1) * P]
                            start = bool(es.tile_start[t])
                            stop = bool(es.tile_stop[t])
                            if layout_b:
                                nc.tensor.matmul(
                                    out=psums[b][:], lhsT=gt, rhs=mm,
                                    start=start, stop=stop,
                                )
                            else:
                                nc.tensor.matmul(
                                    out=psums[b][:], lhsT=mm, rhs=gt,
                                    start=start, stop=stop,
                                )
                        if after_run is not None:
                            after_run(sb, ri)
                    for bi in range(bh):
                        out_cb(b0 + bi, psums[b0 + bi], pre)

            # ---------------- SpMM 1: x1 = L @ x0 (row-major out) --------
            def cb1(b, ps, _pre):
                xb = opool.tile([P, P], BF16, tag="x1st")
                nc.vector.tensor_copy(xb[:], ps[:])
                j = int(np.searchsorted(es.piece_b0, b, side="right")) - 1
                r0 = (b - int(es.piece_b0[j])) * P
                nc.scalar.dma_start(
                    out=x1my[j][r0 : r0 + P, :], in_=xb[:]
                )
                pt = ptpool.tile([P, P], BF16, tag="ptr")
                nc.tensor.transpose(
                    out=pt[:], in_=xb[:], identity=ident_s[:]
                )
                nc.vector.tensor_copy(x1t_s[:, b * P : (b + 1) * P], pt[:])

            def issue_ag(j):
                nc.gpsimd.collective_compute(
                    "AllGather",
                    AluOpType.bypass,
                    replica_groups=[list(range(ncores))],
                    ins=[x1my[j].ap().opt()],
                    outs=[x1p[j].ap().opt()],
                )

            # issue piece-j AllGather one superblock AFTER its last store,
            # so its deps are already resolved when the Pool queue reaches
            # it (a parked collective head-of-line blocks later gathers).
            def after_run1(sb, ri):
                if ri == 0:
                    for j in range(npc - 1):
                        if es.piece_last_sb[j] + 1 == sb:
                            issue_ag(j)

            spmm(x0p, False, cb1, after_run=after_run1)
            issue_ag(npc - 1)

            # -------- SpMM 2 (transposed out) + fused channel mix --------
            def pre_sb2(sb, b0, bh):
                xsb = xpool.tile([P, SB * P], BF16, tag="x0sb")
                nc.sync.dma_start(
                    out=xsb[:, : bh * P].rearrange("p (b f) -> p b f", f=P),
                    in_=x0t[b0 : b0 + bh, :, :].rearrange("b p f -> p b f"),
                )
                return xsb

            def cb2(b, ps, xsb):
                # ps = (L x1)^T block; the -0.5 x0 term is folded into wbd_0
                bi = b % SB
                x0b = xsb[:, bi * P : (bi + 1) * P]
                x2b = opool.tile([P, P], BF16, tag="x2b")
                nc.vector.tensor_copy(x2b[:], ps[:])
                x1tb = x1t_s[:, b * P : (b + 1) * P]
                # channel mix: out = sum_k XkT^T @ Wbd_k (bias on host)
                pm = pmpool.tile([P, P], F32, tag="pmix")
                for k, xk in enumerate((x0b, x1tb, x2b)):
                    nc.tensor.matmul(
                        out=pm[:],
                        lhsT=xk if k != 2 else xk[:],
                        rhs=wbd_s[:, k * P : (k + 1) * P],
                        start=(k == 0),
                        stop=(k == 2),
                    )
                ob = opool.tile([P, P], BF16, tag="ob")
                nc.vector.tensor_copy(ob[:], pm[:])
                nc.scalar.dma_start(
                    out=outp[b * P : (b + 1) * P, :], in_=ob[:]
                )

            # AG for the last piece was issued at the end of phase 1;
            # phase-2 runs consume pieces in order so sb0/ch3 may briefly
            # wait on it, later sbs never do.
            spmm(x1p, True, cb2, pre_sb=pre_sb2)

    nc.compile()
    return nc


# ---------------------------------------------------------------------------
# Host driver
# ---------------------------------------------------------------------------


def prepare(x, weight, bias, lap_vals, lap_rows, lap_cols, ncores=8,
            sb_blocks=3, npc=4):
    x = np.asarray(x, np.float32)
    weight = np.asarray(weight, np.float32)
    bias = np.asarray(bias, np.float32)
    lap_vals = np.asarray(lap_vals, np.float32)
    lap_rows = np.asarray(lap_rows)
    lap_cols = np.asarray(lap_cols)
    B, V, FIN = x.shape
    _, K, FOUT = weight.shape
    assert B == 2 and FIN == 64 and K == 3 and FOUT == 64

    es = EdgeStructure(V, ncores, sb_blocks, npc, lap_rows, lap_cols)

    x0 = np.concatenate([x[0], x[1]], axis=1)  # [V, 128] f32
    # padded per-core [vpad, P] bf16 shards
    xsh = np.zeros((ncores, es.vpad, P), NPBF16)
    for c in range(ncores):
        xsh[c, : es.vsh] = x0[c * es.vsh : (c + 1) * es.vsh].astype(NPBF16)
    # per-piece tables [ncores * Rj*P, P]
    x0p_arrs = []
    for j in range(npc):
        lo, hi = int(es.piece_b0[j]) * P, int(es.piece_b0[j + 1]) * P
        x0p_arrs.append(
            np.ascontiguousarray(xsh[:, lo:hi].reshape(-1, P))
        )

    wbd = np.zeros((3, P, P), np.float32)
    for k in range(3):
        wk = weight[:, k, :] * (2.0 if k == 2 else 1.0)  # x2' = x2/2
        if k == 0:
            wk = wk - weight[:, 2, :]  # fold -0.5 x0 of x2' into x0 term
        wbd[k, :64, :64] = wk
        wbd[k, 64:, 64:] = wk
    wbd = wbd.astype(NPBF16)
    ident = np.eye(P, dtype=np.float32).astype(NPBF16)
    in_maps = []
    for c in range(ncores):
        idx_w, mfull = es.per_core_arrays(c, lap_vals)
        x0t_c = np.ascontiguousarray(
            xsh[c].reshape(es.nblk, P, P).transpose(0, 2, 1)
        )
        im = {
            "x0t": x0t_c,
            "wbd": wbd,
            "ident": ident,
            "eidx": idx_w,
            "emt": mfull,
        }
        for j in range(npc):
            im[f"x0p{j}"] = x0p_arrs[j]
        in_maps.append(im)

    nc = build_program(es)

    def assemble(results):
        out = np.empty((B, V, FOUT), np.float32)
        for c in range(ncores):
            o = np.asarray(results[c]["outp"]).astype(np.float32)
            out[0, c * es.vsh : (c + 1) * es.vsh, :] = o[: es.vsh, :64]
            out[1, c * es.vsh : (c + 1) * es.vsh, :] = o[: es.vsh, 64:]
        return out + bias[None, None, :]

    return nc, in_maps, assemble, es


def kernel(x, weight, bias, lap_vals, lap_rows, lap_cols):
    nc, in_maps, assemble, es = prepare(
        x, weight, bias, lap_vals, lap_rows, lap_cols
    )
    res = bass_utils.run_bass_kernel_spmd(
        nc, in_maps, core_ids=list(range(es.ncores))
    )
    return assemble(res.results)



# revision 2
# speedup vs baseline: 1.0211x; 1.0211x over previous
"""ChebConv (K=3) forward as a distributed Bass/Tile kernel on 8 trn2 NeuronCores.

v3: the SWDGE descriptor generation on GpSimd (Q7) is the serial bottleneck
(~1.7-2ns per gathered row + ~1us fixed per gather instruction), so this
version minimizes exactly that:
  - edges are packed CONTIGUOUSLY into 128-edge tiles per (group=10 dest
    blocks, piece) run; a tile may straddle dest blocks (a straddle costs an
    extra matmul on the underutilized PE instead of index padding on the
    bottleneck GpSimd). Index padding drops from 14% to ~5% (cross-core max
    + ceil only).
  - one dma_gather per run -> 80 gather instructions total instead of 264
    (994ns fixed overhead each).
  - matmuls are issued BLOCK-major (all 4 pieces of dest block b
    consecutively), so only ~2 PSUM accumulators are live at a time even
    though the gather slabs span 10 blocks; PSUM fits in the 8 banks.
  - the selector M[e,j] = val[e] * (lrow[e]==j) is built ON-CHIP by DVE with
    one fused tensor_scalar from an 8B/edge (lrow,val) f32 stream, removing
    the 256B/edge M-matrix DRAM traffic entirely (frees the DMA queues that
    backpressure SWDGE).
  - AllGather of x1 piece j issues right after its last block's store; with
    block-major order, phase 2's first piece-3 gather lands ~50us after
    phase 1 ends, by which time the last AllGather has completed.

Math (same as v1): x0 = [x[0] | x[1]] [V, 128]; x1 = L x0; x2' = L x1 - 0.5 x0
(= x2/2, the 2x folded into W_k2); out = sum_k XkT^T @ Wbd_k with the -x0 W2
term folded into Wbd_0, bias added on host.
"""

import sys

sys.path.insert(0, "/opt/trn_rl_repo")

import numpy as np
import ml_dtypes

import concourse.bass as bass
import concourse.bacc as bacc
import concourse.mybir as mybir
import concourse.tile as tile
from concourse import bass_utils
from concourse.alu_op_type import AluOpType

P = 128
F32 = mybir.dt.float32
BF16 = mybir.dt.bfloat16
I16 = mybir.dt.int16
NPBF16 = ml_dtypes.bfloat16
NQ = 4  # SWDGE queues


def _cdiv(a, b):
    return -(-a // b)


# ---------------------------------------------------------------------------
# Host-side: uniform (cross-core) edge structure + per-core content arrays
# ---------------------------------------------------------------------------


class ES2:
    def __init__(self, V, ncores, G, Rj, rows, cols, vals):
        self.V, self.ncores, self.G = V, ncores, G
        self.vsh = V // ncores                   # 12500
        self.nblk = _cdiv(self.vsh, P)           # 98
        self.vpad = self.nblk * P                # 12544
        self.ngrp = _cdiv(self.nblk, G)          # 10
        npc = len(Rj)
        assert sum(Rj) == self.nblk
        self.npc = npc
        self.Rj = Rj
        self.piece_b0 = np.concatenate(([0], np.cumsum(Rj)))
        self.piece_rows = [r * P for r in Rj]
        assert all(r * ncores * P <= 32768 for r in Rj)

        rows = np.asarray(rows, np.int64)
        cols = np.asarray(cols, np.int64)
        vals = np.asarray(vals, np.float32)

        c_of = cols // self.vsh
        l_of = cols - c_of * self.vsh
        cblk = l_of // P
        piece = np.searchsorted(self.piece_b0, cblk, side="right") - 1
        pr = np.array(self.piece_rows)[piece]
        idx_in_piece = c_of * pr + (l_of - self.piece_b0[piece] * P)
        assert idx_in_piece.max() < 32768

        r_core = rows // self.vsh
        r_loc = rows - r_core * self.vsh
        blk = r_loc // P
        lrow = r_loc % P
        grp = blk // G
        run = grp * npc + piece
        self.nruns = self.ngrp * npc

        counts = np.zeros((ncores, self.nruns), np.int64)
        np.add.at(counts, (r_core, run), 1)
        self.ntr = _cdiv(np.max(counts, axis=0), P)        # tiles per run
        self.tile_base = np.concatenate(([0], np.cumsum(self.ntr)))[:-1]
        self.ntiles = int(np.sum(self.ntr))

        # per-core slot assignment: edges of (core, run) sorted by blk fill
        # the run's tiles contiguously
        order = np.lexsort((blk, run, r_core))  # sort by core, run, blk
        rc_s = r_core[order]
        run_s = run[order]
        key = rc_s * self.nruns + run_s
        starts = np.searchsorted(key, np.arange(ncores * self.nruns),
                                 side="left")
        rank = np.arange(len(order)) - starts[key]
        slot = self.tile_base[run_s] * P + rank
        t_in_run = rank // P

        # matmul entries: union over cores of (run, blk, tile_in_run),
        # sorted so each run's entries are consumed in m-order by the
        # block-major sweep (enables batched wide M builds)
        blk_s = blk[order]
        ekey = (run_s * 128 + blk_s) * 128 + t_in_run
        uk = np.unique(ekey)
        # ensure every block has at least one entry
        ub = np.unique((uk // 128) % 128)
        if len(ub) < self.nblk:
            missing = np.setdiff1d(np.arange(self.nblk), ub)
            extra = []
            for b in missing:
                g = int(b) // G
                for ch in range(npc):
                    r = g * npc + ch
                    if self.ntr[r] > 0:
                        extra.append((r * 128 + int(b)) * 128 + 0)
                        break
                else:
                    raise AssertionError("group with no tiles")
            uk = np.unique(np.concatenate([uk, np.array(extra, np.int64)]))
        self.nm = len(uk)
        ent_run = uk // (128 * 128)
        ent_b = (uk // 128) % 128
        ent_t = uk % 128
        # entry lookup per (run, block): list of (t, m)
        self.entries = {}  # (run, b) -> [(t, m)]
        for m in range(self.nm):
            self.entries.setdefault(
                (int(ent_run[m]), int(ent_b[m])), []).append(
                    (int(ent_t[m]), m))
        # entry column range per group (entries of its runs contiguous in m)
        self.g_m0, self.g_m1 = [], []
        for g in range(self.ngrp):
            lo = np.searchsorted(ent_run, g * npc, side="left")
            hi = np.searchsorted(ent_run, (g + 1) * npc - 1, side="right")
            self.g_m0.append(int(lo))
            self.g_m1.append(int(hi))
        self.max_nm_g = max(b - a for a, b in zip(self.g_m0, self.g_m1))
        # per-run m range (contiguous) for lazy wide M builds
        self.run_m0 = [int(np.searchsorted(ent_run, r, side="left"))
                       for r in range(self.nruns)]
        self.run_m1 = [int(np.searchsorted(ent_run, r, side="right"))
                       for r in range(self.nruns)]
        # hybrid M sourcing: chunk (run, ci) of MK entries is either built
        # on DVE or streamed from a packed DRAM blob. ~60% DVE keeps DVE
        # under the per-phase GpSimd span; the rest rides idle DMA capacity.
        MK = 8
        self.MK = MK
        self.chunk_blob = {}   # (run, ci) -> (blob col base, k)
        w = 0
        for r in range(self.nruns):
            r0, r1 = self.run_m0[r], self.run_m1[r]
            nch = _cdiv(r1 - r0, MK)
            for ci in range(nch):
                if (r + ci) % 8 >= 3:  # DRAM-sourced chunk
                    k = min(MK, r1 - r0 - ci * MK)
                    self.chunk_blob[(r, ci)] = (w, k)
                    w += k * P
        self.blob_w = max(w, P)

        m_of_edge = np.searchsorted(uk, ekey)
        assert np.array_equal(uk[m_of_edge], ekey)

        # stash per-core data
        self._percore = []
        for c in range(ncores):
            sel = rc_s == c
            self._percore.append((slot[sel], m_of_edge[sel],
                                  lrow[order][sel],
                                  idx_in_piece[order][sel],
                                  vals[order][sel]))
        self.GW = int(np.max(self.ntr))
        self.pad_frac = self.ntiles * P / (len(rows) / ncores) - 1

    def grp_blocks(self, g):
        return range(g * self.G, min((g + 1) * self.G, self.nblk))

    def entry_chunk(self, run, m):
        return (m - self.run_m0[run]) // self.MK

    def per_core_arrays(self, core):
        slot, m_of, lrow, idxp, vals = self._percore[core]
        n = self.ntiles * P
        idx = np.zeros(n, np.int16)
        idx[slot] = idxp.astype(np.int16)
        idx_w = np.tile(np.ascontiguousarray(idx.reshape(-1, 16).T), (8, 1))
        lrv = np.zeros((P, 2 * self.nm), np.float32)
        part = slot % P
        lrv[part, m_of] = lrow                 # lr plane: cols [0, nm)
        lrv[part, self.nm + m_of] = vals       # vv plane: cols [nm, 2nm)
        # packed DRAM M blob for the DRAM-sourced chunks
        ent_run_of = np.zeros(self.nm, np.int64)
        for r in range(self.nruns):
            ent_run_of[self.run_m0[r]:self.run_m1[r]] = r
        er = ent_run_of[m_of]
        eci = (m_of - np.array(self.run_m0)[er]) // self.MK
        blob = np.zeros((P, self.blob_w), NPBF16)
        base = np.full(len(m_of), -1, np.int64)
        for (r, ci), (b0, k) in self.chunk_blob.items():
            pass
        # vectorized base lookup
        bmap = {}
        for (r, ci), (b0, k) in self.chunk_blob.items():
            bmap[(r, ci)] = b0
        keys = er * 1000 + eci
        ukeys, inv = np.unique(keys, return_inverse=True)
        ubase = np.array([bmap.get((int(kk) // 1000, int(kk) % 1000), -1)
                          for kk in ukeys], np.int64)
        base = ubase[inv]
        sel = base >= 0
        m_lo = np.array(self.run_m0)[er] + eci * self.MK
        col = base + (m_of - m_lo) * P + lrow
        blob[part[sel], col[sel]] = vals[sel].astype(NPBF16)
        return idx_w, lrv.astype(NPBF16), blob


# ---------------------------------------------------------------------------
# Bass program (SPMD: one program, per-core data via in_maps)
# ---------------------------------------------------------------------------


def build_program(es: ES2):
    nblk, vpad, ncores = es.nblk, es.vpad, es.ncores
    G, npc, GW = es.G, es.npc, es.GW
    MK = 8  # M tiles per wide DVE build

    nc = bacc.Bacc(
        "TRN2",
        target_bir_lowering=False,
        debug=False,
        num_devices=ncores,
        num_swdge_queues=NQ,
    )

    x0p = [
        nc.dram_tensor(f"x0p{j}", [ncores * es.piece_rows[j], P], BF16,
                       kind="ExternalInput")
        for j in range(npc)
    ]
    x0t = nc.dram_tensor("x0t", [nblk, P, P], BF16, kind="ExternalInput")
    wbd = nc.dram_tensor("wbd", [3, P, P], BF16, kind="ExternalInput")
    ident_d = nc.dram_tensor("ident", [P, P], BF16, kind="ExternalInput")
    iota_d = nc.dram_tensor("iota", [P, MK * P], BF16, kind="ExternalInput")
    eidx = nc.dram_tensor("eidx", [P, es.ntiles * 8], I16,
                          kind="ExternalInput")
    lrv_d = nc.dram_tensor("lrv", [P, 2 * es.nm], BF16, kind="ExternalInput")
    mblob = nc.dram_tensor("mblob", [P, es.blob_w], BF16,
                           kind="ExternalInput")
    outp = nc.dram_tensor("outp", [vpad, P], BF16, kind="ExternalOutput")

    x1my = [
        nc.dram_tensor(f"x1my{j}", [es.piece_rows[j], P], BF16)
        for j in range(npc)
    ]
    x1p = [
        nc.dram_tensor(f"x1p{j}", [ncores * es.piece_rows[j], P], BF16,
                       addr_space="Shared")
        for j in range(npc)
    ]

    # block -> AllGather piece to issue right after its drain (phase 1);
    # the LAST piece's AG is deferred into phase 2 (its trigger parks the
    # gpsimd queue until the collective completes)
    ag_after_block = {int(es.piece_b0[j + 1]) - 1: j for j in range(npc - 1)}

    def make_startstop():
        """block-major issue order: per block, first/last entry (run,t,m)."""
        starts, stops = set(), set()
        for g in range(es.ngrp):
            for b in es.grp_blocks(g):
                keys = []
                for ch in range(npc):
                    run = g * npc + ch
                    for (t, m) in es.entries.get((run, b), []):
                        keys.append((run, t, m))
                assert keys, f"block {b} has no entries"
                starts.add(keys[0])
                stops.add(keys[-1])
        return starts, stops

    with tile.TileContext(nc) as tc:
        with (
            tc.tile_pool(name="const", bufs=1) as cpool,
            tc.tile_pool(name="lstage", bufs=3) as lpool,
            tc.tile_pool(name="gslab", bufs=13) as gpool,
            tc.tile_pool(name="mslab", bufs=6) as mpool,
            tc.tile_pool(name="mdram", bufs=16) as mdpool,
            tc.tile_pool(name="xio", bufs=2) as xpool,
            tc.tile_pool(name="ostage", bufs=8) as opool,
            tc.tile_pool(name="acc", bufs=6, space="PSUM") as apool,
            tc.tile_pool(name="aux", bufs=2, space="PSUM") as auxpool,
        ):
            ident_s = cpool.tile([P, P], BF16, tag="ident")
            nc.sync.dma_start(out=ident_s[:], in_=ident_d[:, :])
            iota_s = cpool.tile([P, MK * P], BF16, tag="iota")
            nc.sync.dma_start(out=iota_s[:], in_=iota_d[:, :])
            wbd_s = cpool.tile([P, 3 * P], BF16, tag="wbd")
            for k in range(3):
                nc.sync.dma_start(
                    out=wbd_s[:, k * P:(k + 1) * P], in_=wbd[k, :, :]
                )
            x1t_s = cpool.tile([P, nblk * P], BF16, tag="x1t")
            eidx_s = cpool.tile([P, es.ntiles * 8], I16, tag="eidx")
            nc.sync.dma_start(out=eidx_s[:], in_=eidx[:, :])

            qn = [0]
            starts, stops = make_startstop()

            def issue_ag(j):
                nc.gpsimd.collective_compute(
                    "AllGather",
                    AluOpType.bypass,
                    replica_groups=[list(range(ncores))],
                    ins=[x1my[j].ap().opt()],
                    outs=[x1p[j].ap().opt()],
                )

            SUBT = (GW + 1) // 2  # tiles per sub-slab (2 subs per run)

            def emit_spmm(src_tabs, layout_b, out_cb, pre_g=None,
                          post_block=None, gather_stream=None):
                # gather_stream: ordered list of ('gather', g, ch) and
                # ('ag', j) items; default = group-major chunk order
                if gather_stream is None:
                    gather_stream = [("gather", g, ch)
                                     for g in range(es.ngrp)
                                     for ch in range(npc)]
                slabs = {}
                spt = [0]
                emitted = set()

                def emit_gathers_until(g_done):
                    # emit stream items until all of group g_done's gathers
                    # are out
                    def pending():
                        return any((g_done, ch) not in emitted
                                   for ch in range(npc)
                                   if es.ntr[g_done * npc + ch] > 0)
                    while pending():
                        item = gather_stream[spt[0]]
                        spt[0] += 1
                        if item[0] == "ag":
                            issue_ag(item[1])
                            continue
                        _, g, ch = item
                        run = g * npc + ch
                        ntr = int(es.ntr[run])
                        emitted.add((g, ch))
                        if ntr == 0:
                            continue
                        t0 = int(es.tile_base[run])
                        for su in range(_cdiv(ntr, SUBT)):
                            tl = su * SUBT
                            th_ = min(ntr, tl + SUBT)
                            slab = gpool.tile([P, SUBT * P], BF16, tag="g")
                            nidx = (th_ - tl) * P
                            nc.gpsimd.dma_gather(
                                out_ap=slab[:, :nidx].rearrange(
                                    "p (t e) -> p t e", e=P),
                                in_ap=src_tabs[ch][:, :],
                                idxs_ap=eidx_s[:, 8 * (t0 + tl):
                                               8 * (t0 + th_)],
                                num_idxs=nidx,
                                num_idxs_reg=nidx,
                                elem_size=P,
                                single_packet=False,
                                queue_num=qn[0] % NQ,
                            )
                            qn[0] += 1
                            slabs[(g, ch, su)] = slab

                for g in range(es.ngrp):
                    emit_gathers_until(g)
                    lv = lpool.tile([P, 2 * es.max_nm_g], BF16, tag="lrv")
                    m0, m1 = es.g_m0[g], es.g_m1[g]
                    nmg = m1 - m0
                    nc.sync.dma_start(out=lv[:, :nmg],
                                      in_=lrv_d[:, m0:m1])
                    nc.sync.dma_start(out=lv[:, nmg:2 * nmg],
                                      in_=lrv_d[:, es.nm + m0:es.nm + m1])
                    pre = pre_g(g) if pre_g is not None else None
                    # hybrid M: DVE-built wide chunks (lazy, in m-order) or
                    # DRAM-streamed wide chunks (pre-issued at group start)
                    wides = {}   # (run, chunk) -> wide tile
                    for ch in range(npc):
                        run = g * npc + ch
                        r0, r1 = es.run_m0[run], es.run_m1[run]
                        for ci in range(_cdiv(r1 - r0, MK)):
                            cb = es.chunk_blob.get((run, ci))
                            if cb is None:
                                continue
                            b0, k = cb
                            wt = mdpool.tile([P, MK * P], BF16, tag="md")
                            nc.sync.dma_start(
                                out=wt[:, :k * P],
                                in_=mblob[:, b0:b0 + k * P])
                            wides[(run, ci)] = wt
                    def get_m(run, m):
                        r0 = es.run_m0[run]
                        ci = (m - r0) // MK
                        wt = wides.get((run, ci))
                        if wt is None:
                            lo = r0 + ci * MK
                            k = min(MK, es.run_m1[run] - lo)
                            wt = mpool.tile([P, MK * P], BF16, tag="mw")
                            lr_ap = lv[:, lo - m0:lo - m0 + k].rearrange(
                                "p (k o) -> p k o", o=1).to_broadcast(
                                [P, k, P])
                            vv_ap = lv[:, nmg + (lo - m0):
                                       nmg + (lo - m0) + k].rearrange(
                                "p (k o) -> p k o", o=1).to_broadcast(
                                [P, k, P])
                            wv = wt[:, :k * P].rearrange(
                                "p (k f) -> p k f", f=P)
                            nc.vector.tensor_tensor(
                                out=wv, in0=iota_s[:, :k * P].rearrange(
                                    "p (k f) -> p k f", f=P),
                                in1=lr_ap, op=AluOpType.is_equal)
                            nc.vector.tensor_tensor(
                                out=wv, in0=wv, in1=vv_ap,
                                op=AluOpType.mult)
                            wides[(run, ci)] = wt
                        off = ((m - r0) % MK) * P
                        return wt[:, off:off + P]
                    for b in es.grp_blocks(g):
                        acc = apool.tile([P, P], F32, tag="acc",
                                         name=f"acc{b}")
                        for ch in range(npc):
                            run = g * npc + ch
                            for (t, m) in es.entries.get((run, b), []):
                                mt = get_m(run, m)
                                gt = slabs[(g, ch, t // SUBT)][
                                    :, (t % SUBT) * P:(t % SUBT + 1) * P]
                                key = (run, t, m)
                                st, sp = key in starts, key in stops
                                if layout_b:
                                    nc.tensor.matmul(
                                        out=acc[:], lhsT=gt, rhs=mt,
                                        start=st, stop=sp)
                                else:
                                    nc.tensor.matmul(
                                        out=acc[:], lhsT=mt, rhs=gt,
                                        start=st, stop=sp)
                        out_cb(b, acc, pre)
                        if post_block is not None:
                            post_block(b)

            # ---------------- SpMM 1: x1 = L @ x0 (row-major out) --------
            def cb1(b, ps, _pre):
                xb = opool.tile([P, P], BF16, tag="x1st")
                nc.vector.tensor_copy(xb[:], ps[:])
                j = int(np.searchsorted(es.piece_b0, b, side="right")) - 1
                r0 = (b - int(es.piece_b0[j])) * P
                nc.scalar.dma_start(out=x1my[j][r0:r0 + P, :], in_=xb[:])
                pt = auxpool.tile([P, P], BF16, tag="aux")
                nc.tensor.transpose(out=pt[:], in_=xb[:],
                                    identity=ident_s[:])
                nc.vector.tensor_copy(x1t_s[:, b * P:(b + 1) * P], pt[:])

            def post_block1(b):
                if b in ag_after_block:
                    issue_ag(ag_after_block[b])

            emit_spmm(x0p, False, cb1, post_block=post_block1)

            # -------- SpMM 2 (transposed out) + fused channel mix --------
            def pre_g2(g):
                b0 = g * G
                bh = len(es.grp_blocks(g))
                xsb = xpool.tile([P, G * P], BF16, tag="x0sb")
                nc.sync.dma_start(
                    out=xsb[:, : bh * P].rearrange("p (b f) -> p b f", f=P),
                    in_=x0t[b0:b0 + bh, :, :].rearrange("b p f -> p b f"),
                )
                return xsb

            def cb2(b, ps, xsb):
                bi = b % G
                x0b = xsb[:, bi * P:(bi + 1) * P]
                x2b = opool.tile([P, P], BF16, tag="x2b")
                nc.vector.tensor_copy(x2b[:], ps[:])
                x1tb = x1t_s[:, b * P:(b + 1) * P]
                pm = auxpool.tile([P, P], F32, tag="aux")
                for k, xk in enumerate((x0b, x1tb, x2b)):
                    nc.tensor.matmul(
                        out=pm[:], lhsT=xk if k != 2 else xk[:],
                        rhs=wbd_s[:, k * P:(k + 1) * P],
                        start=(k == 0), stop=(k == 2),
                    )
                ob = opool.tile([P, P], BF16, tag="ob")
                nc.vector.tensor_copy(ob[:], pm[:])
                nc.scalar.dma_start(out=outp[b * P:(b + 1) * P, :],
                                    in_=ob[:])

            stream2 = [("gather", 0, 0), ("gather", 0, 1),
                       ("gather", 1, 0), ("gather", 1, 1),
                       ("gather", 0, 2), ("ag", npc - 1),
                       ("gather", 0, 3), ("gather", 1, 2),
                       ("gather", 1, 3)]
            stream2 += [("gather", g, ch) for g in range(2, es.ngrp)
                        for ch in range(npc)]
            emit_spmm(x1p, True, cb2, pre_g=pre_g2,
                      gather_stream=stream2)

    nc.compile()
    return nc


# ---------------------------------------------------------------------------
# Host driver
# ---------------------------------------------------------------------------


def prepare(x, weight, bias, lap_vals, lap_rows, lap_cols, ncores=8,
            G=10, Rj=(27, 27, 27, 17)):
    x = np.asarray(x, np.float32)
    weight = np.asarray(weight, np.float32)
    bias = np.asarray(bias, np.float32)
    lap_vals = np.asarray(lap_vals, np.float32)
    lap_rows = np.asarray(lap_rows)
    lap_cols = np.asarray(lap_cols)
    B, V, FIN = x.shape
    _, K, FOUT = weight.shape
    assert B == 2 and FIN == 64 and K == 3 and FOUT == 64

    es = ES2(V, ncores, G, list(Rj), lap_rows, lap_cols, lap_vals)

    x0 = np.concatenate([x[0], x[1]], axis=1)  # [V, 128] f32
    xsh = np.zeros((ncores, es.vpad, P), NPBF16)
    for c in range(ncores):
        xsh[c, : es.vsh] = x0[c * es.vsh:(c + 1) * es.vsh].astype(NPBF16)
    x0p_arrs = []
    for j in range(es.npc):
        lo, hi = int(es.piece_b0[j]) * P, int(es.piece_b0[j + 1]) * P
        x0p_arrs.append(np.ascontiguousarray(xsh[:, lo:hi].reshape(-1, P)))

    wbd = np.zeros((3, P, P), np.float32)
    for k in range(3):
        wk = weight[:, k, :] * (2.0 if k == 2 else 1.0)  # x2' = x2/2
        if k == 0:
            wk = wk - weight[:, 2, :]  # fold -0.5 x0 of x2' into x0 term
        wbd[k, :64, :64] = wk
        wbd[k, 64:, 64:] = wk
    wbd = wbd.astype(NPBF16)
    ident = np.eye(P, dtype=np.float32).astype(NPBF16)
    MK = 8
    iota = np.ascontiguousarray(np.tile(
        np.arange(P, dtype=np.float32)[None, :],
        (P, MK)).astype(NPBF16))

    in_maps = []
    for c in range(ncores):
        idx_w, lrv, blob = es.per_core_arrays(c)
        x0t_c = np.ascontiguousarray(
            xsh[c].reshape(es.nblk, P, P).transpose(0, 2, 1)
        )
        im = {
            "x0t": x0t_c,
            "wbd": wbd,
            "ident": ident,
            "iota": iota,
            "eidx": idx_w,
            "lrv": lrv,
            "mblob": blob,
        }
        for j in range(es.npc):
            im[f"x0p{j}"] = x0p_arrs[j]
        in_maps.append(im)

    nc = build_program(es)

    def assemble(results):
        out = np.empty((B, V, FOUT), np.float32)
        for c in range(ncores):
            o = np.asarray(results[c]["outp"]).astype(np.float32)
            out[0, c * es.vsh:(c + 1) * es.vsh, :] = o[: es.vsh, :64]
            out[1, c * es.vsh:(c + 1) * es.vsh, :] = o[: es.vsh, 64:]
        return out + bias[None, None, :]

    return nc, in_maps, assemble, es


def kernel(x, weight, bias, lap_vals, lap_rows, lap_cols):
    nc, in_maps, assemble, es = prepare(
        x, weight, bias, lap_vals, lap_rows, lap_cols
    )
    res = bass_utils.run_bass_kernel_spmd(
        nc, in_maps, core_ids=list(range(es.ncores))
    )
    return assemble(res.results)
